# revision 9
# baseline (speedup 1.0000x reference)
"""Trainium2 Bass kernel for CustomMultiHeadAttention (B=2, L=2048, D=512, H=8).

Sharding: 8 cores = 2 batches x 4 head-pairs. Each core computes, for its
batch b and its 2 heads, the partial output (O_h @ Wo_h summed over its
heads), transposed: poutT [512, 2048]. Host sums the 4 partials per batch,
transposes, and adds bo.

Device-side math per core (all masking folded into matmul contractions):
  Qh = (q[b]*qm) @ WqT_cols + qm*bq_cols          (masked q rows -> exactly 0)
  Kh = k[b] @ WkT_cols/8 + bk_cols/8
  E[k,q] = Kh.Qh + (kb[k]-c)*qm[q] + c            via 2 extra contraction rows
           (kb = -1e4 for masked keys, c = ln(1/2048))
    -> unmasked q: E = s + kb  (masked keys underflow to 0 in exp)
    -> masked q:   E = c       (exp = 1/2048 uniform; denom = 1)
  PT = exp(E)   [k, q] layout
  outT = [Vp | 1]^T @ PT   (Vp = coef * Vh; ones column yields denom row)
  O = outT[0:64] / outT[64]
  poutT[d, q] += Wo_h[:, d] . O[:, q]
"""

import math
import os

os.environ.setdefault("MYCRO_LOCAL_CACHE", "1")

import numpy as np

import concourse.bass as bass
import concourse.tile as tile
from concourse import bacc
from concourse import mybir
from concourse.bass_utils import run_bass_kernel_spmd
from concourse.masks import make_identity

B = 2
L = 2048
DM = 512
H = 8
DH = 64
NCORES = 8
HPC = 2           # heads per core
DH2 = HPC * DH    # 128
NKT = L // 128    # 16 k tiles
QH = 1024         # q chunk for attention phase
NQH = L // QH     # 2
C_LN = -math.log(L)
NEG = -10000.0

F32 = mybir.dt.float32
F32R = mybir.dt.float32r
BF16 = mybir.dt.bfloat16

# dtype for exp output (PT) and V' — bf16 halves SBUF and enables fast PV
ATT_DT = BF16
# dtype for PE-streamed f32 operands: F32R = full-rate, F32 = 4x slower exact
MM_DT = F32R

TRACE = False
LAST_RESULT = None

AUX_QM, AUX_KBMC, AUX_ONES, AUX_CLN = 0, 1, 2, 3


def build_nc():
    nc = bacc.Bacc(None, target_bir_lowering=False)

    xqT_d = nc.declare_dram_parameter("xqT", [DM, L], MM_DT, isOutput=False)
    xkT_d = nc.declare_dram_parameter("xkT", [DM, L], MM_DT, isOutput=False)
    xvT_d = nc.declare_dram_parameter("xvT", [DM, L], MM_DT, isOutput=False)
    wqs_d = nc.declare_dram_parameter("wqs", [DM, DH2], MM_DT, isOutput=False)
    wks_d = nc.declare_dram_parameter("wks", [DM, DH2], MM_DT, isOutput=False)
    wvs_d = nc.declare_dram_parameter("wvs", [DM, DH2], MM_DT, isOutput=False)
    wbias_d = nc.declare_dram_parameter("wbias", [1, 4 * DH2], MM_DT, isOutput=False)
    wos_d = nc.declare_dram_parameter("wos", [DH2, DM], MM_DT, isOutput=False)
    aux_d = nc.declare_dram_parameter("aux", [4, L], MM_DT, isOutput=False)
    coef_d = nc.declare_dram_parameter("coef", [1, L], F32, isOutput=False)
    pout_d = nc.declare_dram_parameter("poutT", [DM, L], F32, isOutput=True)

    with tile.TileContext(nc) as tc:
        with tc.tile_pool(name="const", bufs=1) as const:
            ident = const.tile([128, 128], F32)
            make_identity(nc, ident)

            wq_sb = const.tile([128, 4, DH2], MM_DT)
            nc.sync.dma_start(
                out=wq_sb, in_=wqs_d[:, :].rearrange("(t p) m -> p t m", p=128)
            )
            wk_sb = const.tile([128, 4, DH2], MM_DT)
            nc.sync.dma_start(
                out=wk_sb, in_=wks_d[:, :].rearrange("(t p) m -> p t m", p=128)
            )
            wv_sb = const.tile([128, 4, DH2], MM_DT)
            nc.sync.dma_start(
                out=wv_sb, in_=wvs_d[:, :].rearrange("(t p) m -> p t m", p=128)
            )
            wb_sb = const.tile([1, 4 * DH2], MM_DT)
            nc.sync.dma_start(out=wb_sb, in_=wbias_d[:, :])
            wo_sb = [const.tile([DH, DM], MM_DT, name=f"wo{h}") for h in range(HPC)]
            for h in range(HPC):
                nc.sync.dma_start(out=wo_sb[h], in_=wos_d[h * DH : (h + 1) * DH, :])
            aux_sb = const.tile([1, 4 * L], MM_DT)
            nc.sync.dma_start(
                out=aux_sb,
                in_=aux_d[:, :]
                .rearrange("a b -> (a b)")
                .rearrange("(o ab) -> o ab", o=1),
            )
            coef_sb = const.tile([128, NKT], F32)
            nc.sync.dma_start(
                out=coef_sb,
                in_=coef_d[0:1, :].rearrange("1 (t p) -> p t", p=128),
            )

            with tc.tile_pool(name="qek", bufs=1) as qek:
                QE = [qek.tile([66, L], MM_DT, name=f"QE{h}") for h in range(HPC)]
                KE = [qek.tile([66, L], MM_DT, name=f"KE{h}") for h in range(HPC)]
                Vp = [
                    qek.tile([128, NKT, DH + 1], ATT_DT, name=f"Vp{h}")
                    for h in range(HPC)
                ]

                # ---------------- Phase A: projections ----------------
                with (
                    tc.tile_pool(name="xin", bufs=5) as xin,
                    tc.tile_pool(name="psA", bufs=2, space="PSUM") as psA,
                    tc.tile_pool(name="vtmp", bufs=1) as vtmp,
                ):
                    VT_sb = vtmp.tile([128, L], F32)

                    projs = [
                        ("q", xqT_d, wq_sb, 0, AUX_QM),
                        ("k", xkT_d, wk_sb, 1, AUX_ONES),
                        ("v", xvT_d, wv_sb, 2, AUX_ONES),
                    ]
                    for pname, xdram, w_sb, brow, auxrow in projs:
                        xts = []
                        for t in range(4):
                            xt = xin.tile(
                                [128, L], MM_DT, tag="xin", name=f"x{pname}{t}"
                            )
                            nc.sync.dma_start(
                                out=xt, in_=xdram[t * 128 : (t + 1) * 128, :]
                            )
                            xts.append(xt)
                        for ch in range(4):
                            sl = slice(ch * 512, (ch + 1) * 512)
                            ps = psA.tile([128, 512], F32, tag="proj", name="ps")
                            for t in range(4):
                                nc.tensor.matmul(
                                    ps,
                                    lhsT=w_sb[:, t, :],
                                    rhs=xts[t][:, sl],
                                    start=(t == 0),
                                    stop=False,
                                )
                            nc.tensor.matmul(
                                ps,
                                lhsT=wb_sb[0:1, brow * DH2 : (brow + 1) * DH2],
                                rhs=aux_sb[
                                    0:1,
                                    auxrow * L + ch * 512 : auxrow * L + (ch + 1) * 512,
                                ],
                                start=False,
                                stop=True,
                            )
                            if pname == "q":
                                for h in range(HPC):
                                    nc.vector.tensor_copy(
                                        out=QE[h][0:DH, sl],
                                        in_=ps[h * DH : (h + 1) * DH, :],
                                    )
                            elif pname == "k":
                                for h in range(HPC):
                                    nc.vector.tensor_copy(
                                        out=KE[h][0:DH, sl],
                                        in_=ps[h * DH : (h + 1) * DH, :],
                                    )
                            else:
                                nc.vector.tensor_copy(out=VT_sb[:, sl], in_=ps)

                    # V: transpose [dh2, k] -> [k, dh2], scale by coef
                    for kt in range(NKT):
                        tp = psA.tile([128, 128], F32, tag="tp", name="tp")
                        nc.tensor.transpose(
                            tp, VT_sb[:, kt * 128 : (kt + 1) * 128], ident
                        )
                        for h in range(HPC):
                            nc.vector.tensor_scalar_mul(
                                out=Vp[h][:, kt, 0:DH],
                                in0=tp[:, h * DH : (h + 1) * DH],
                                scalar1=coef_sb[:, kt : kt + 1],
                            )
                    for h in range(HPC):
                        nc.vector.memset(Vp[h][:, :, DH : DH + 1], 1.0)
                        nc.sync.dma_start(
                            out=QE[h][64:65, :], in_=aux_d[AUX_QM : AUX_QM + 1, :]
                        )
                        nc.sync.dma_start(
                            out=QE[h][65:66, :], in_=aux_d[AUX_ONES : AUX_ONES + 1, :]
                        )
                        nc.sync.dma_start(
                            out=KE[h][64:65, :], in_=aux_d[AUX_KBMC : AUX_KBMC + 1, :]
                        )
                        nc.sync.dma_start(
                            out=KE[h][65:66, :], in_=aux_d[AUX_CLN : AUX_CLN + 1, :]
                        )

                # ---------------- Phase B: attention ----------------
                with (
                    tc.tile_pool(name="ptp", bufs=2) as ptp,
                    tc.tile_pool(name="psB", bufs=1, space="PSUM") as psB,
                    tc.tile_pool(name="sbB", bufs=2) as sbB,
                ):
                    for qh in range(NQH):
                        nrm = []
                        for h in range(HPC):
                            pt = ptp.tile([128, NKT, QH], ATT_DT, tag="pt", name="pt")
                            for kt in range(NKT):
                                st = psB.tile(
                                    [128, QH], F32, tag="st", bufs=2, name="st"
                                )
                                for c2 in range(QH // 512):
                                    nc.tensor.matmul(
                                        st[:, c2 * 512 : (c2 + 1) * 512],
                                        lhsT=KE[h][0:66, kt * 128 : (kt + 1) * 128],
                                        rhs=QE[h][
                                            0:66,
                                            qh * QH + c2 * 512 : qh * QH
                                            + (c2 + 1) * 512,
                                        ],
                                        start=True,
                                        stop=True,
                                    )
                                nc.scalar.activation(
                                    out=pt[:, kt, :],
                                    in_=st,
                                    func=mybir.ActivationFunctionType.Exp,
                                )
                            outp = psB.tile(
                                [65, QH], F32, tag="outp", bufs=1, name="outp"
                            )
                            for c2 in range(QH // 512):
                                for kt in range(NKT):
                                    nc.tensor.matmul(
                                        outp[:, c2 * 512 : (c2 + 1) * 512],
                                        lhsT=Vp[h][:, kt, :],
                                        rhs=pt[:, kt, c2 * 512 : (c2 + 1) * 512],
                                        start=(kt == 0),
                                        stop=(kt == NKT - 1),
                                    )
                            # normalize: O = outp[0:64] / outp[64]
                            outsb = sbB.tile([65, QH], F32, tag="outsb", name="outsb")
                            nc.vector.tensor_copy(out=outsb, in_=outp)
                            rcp = sbB.tile([1, QH], F32, tag="rcp", name="rcp")
                            nc.vector.reciprocal(out=rcp, in_=outsb[64:65, :])
                            rbc = sbB.tile([DH, QH], F32, tag="rbc", name="rbc")
                            nc.gpsimd.partition_broadcast(rbc, rcp[0:1, :], channels=DH)
                            nr = sbB.tile(
                                [DH, QH], MM_DT, tag=f"nrm{h}", name=f"nr{h}"
                            )
                            nc.vector.tensor_mul(out=nr, in0=outsb[0:DH, :], in1=rbc)
                            nrm.append(nr)
                        # final projection for this q chunk, both heads
                        for dt4 in range(4):
                            for c2 in range(QH // 512):
                                fin = psB.tile(
                                    [128, 512], F32, tag="fin", bufs=1, name="fin"
                                )
                                for h in range(HPC):
                                    nc.tensor.matmul(
                                        fin,
                                        lhsT=wo_sb[h][:, dt4 * 128 : (dt4 + 1) * 128],
                                        rhs=nrm[h][:, c2 * 512 : (c2 + 1) * 512],
                                        start=(h == 0),
                                        stop=(h == HPC - 1),
                                    )
                                fsb = sbB.tile([128, 512], F32, tag="fsb", name="fsb")
                                nc.vector.tensor_copy(out=fsb, in_=fin)
                                nc.sync.dma_start(
                                    out=pout_d[
                                        dt4 * 128 : (dt4 + 1) * 128,
                                        qh * QH + c2 * 512 : qh * QH + (c2 + 1) * 512,
                                    ],
                                    in_=fsb,
                                )
    nc.compile()
    return nc


_CACHE = {}


def _get_nc():
    if "nc" not in _CACHE:
        _CACHE["nc"] = build_nc()
    return _CACHE["nc"]


def kernel(q, k, v, text_mask, audio_mask, n_head, wq, bq, wk, bk, wv, bv, wo, bo):
    global LAST_RESULT
    q = np.asarray(q, np.float32)
    k = np.asarray(k, np.float32)
    v = np.asarray(v, np.float32)
    text_mask = np.asarray(text_mask, np.float32)
    audio_mask = np.asarray(audio_mask, np.float32)
    wq = np.asarray(wq, np.float32)
    wk = np.asarray(wk, np.float32)
    wv = np.asarray(wv, np.float32)
    wo = np.asarray(wo, np.float32)
    bq = np.asarray(bq, np.float32)
    bk = np.asarray(bk, np.float32)
    bv = np.asarray(bv, np.float32)
    bo = np.asarray(bo, np.float32)
    assert int(n_head) == H

    pad = np.concatenate([text_mask, audio_mask], axis=1)  # [B, L]
    qm = (pad != 0).astype(np.float32)
    tl = text_mask.sum(1)
    al = audio_mask.sum(1)
    tot = tl + al
    coef = np.concatenate(
        [
            text_mask * (tot / (2.0 * tl))[:, None],
            audio_mask * (tot / (2.0 * al))[:, None],
        ],
        axis=1,
    ).astype(np.float32)
    kbmc = (NEG * (1.0 - qm) - C_LN).astype(np.float32)
    ones_row = np.ones((L,), np.float32)
    cln_row = np.full((L,), C_LN, np.float32)

    def cc(a):
        return np.ascontiguousarray(a, dtype=np.float32)

    in_maps = []
    for core in range(NCORES):
        b, hp = divmod(core, NCORES // B)
        cols = slice(hp * DH2, (hp + 1) * DH2)
        in_maps.append(
            {
                "xqT": cc((q[b] * qm[b][:, None]).T),
                "xkT": cc(k[b].T),
                "xvT": cc(v[b].T),
                "wqs": cc(wq.T[:, cols]),
                "wks": cc(wk.T[:, cols] / 8.0),
                "wvs": cc(wv.T[:, cols]),
                "wbias": cc(
                    np.concatenate(
                        [bq[cols], bk[cols] / 8.0, bv[cols], np.zeros(DH2, np.float32)]
                    )
                ).reshape(1, 4 * DH2),
                "wos": cc(wo.T[cols, :]),
                "aux": cc(np.stack([qm[b], kbmc[b], ones_row, cln_row])),
                "coef": cc(coef[b]).reshape(1, L),
            }
        )

    res = run_bass_kernel_spmd(
        _get_nc(), in_maps, core_ids=list(range(NCORES)), trace=TRACE
    )
    LAST_RESULT = res

    out = np.zeros((B, L, DM), np.float32)
    npc = NCORES // B
    for b in range(B):
        acc = res.results[b * npc]["poutT"].astype(np.float32).copy()
        for hp in range(1, npc):
            acc += res.results[b * npc + hp]["poutT"]
        out[b] = acc.T + bo[None, :]
    return out


# revision 13
# speedup vs baseline: 1.1609x; 1.1609x over previous
"""Trainium2 Bass kernel for CustomMultiHeadAttention (B=2, L=2048, D=512, H=8).

Sharding: 8 cores = 2 batches x 4 head-pairs. Each core computes, for its
batch b and its 2 heads, the partial output (O_h @ Wo_h summed over its
heads), transposed: poutT [512, 2048]. Host sums the 4 partials per batch,
transposes, and adds bo.

Device-side math per core (all masking folded into matmul contractions):
  Qh = (q[b]*qm) @ WqT_cols + qm*bq_cols          (masked q rows -> exactly 0)
  Kh = k[b] @ WkT_cols/8 + bk_cols/8
  E[k,q] = Kh.Qh + (kb[k]-c)*qm[q] + c            via 2 extra contraction rows
           (kb = -1e4 for masked keys, c = ln(1/2048))
    -> unmasked q: E = s + kb  (masked keys underflow to 0 in exp)
    -> masked q:   E = c       (exp = 1/2048 uniform; denom = 1)
  PT = exp(E)   [k, q] layout
  outT = [Vp | 1]^T @ PT   (Vp = coef * Vh; ones column yields denom row)
  O = outT[0:64] / outT[64]
  poutT[d, q] += Wo_h[:, d] . O[:, q]

Emission order keeps the PE dense (HAM warm): q-proj, k-proj, first
scores+exp unit, then v-proj/transposes, then the remaining attention units.
"""

import math
import os

os.environ.setdefault("MYCRO_LOCAL_CACHE", "1")

import numpy as np

import concourse.bass as bass
import concourse.tile as tile
from concourse import bacc
from concourse import mybir
from concourse.bass_utils import run_bass_kernel_spmd
from concourse.masks import make_identity

B = 2
L = 2048
DM = 512
H = 8
DH = 64
NCORES = 8
HPC = 2           # heads per core
DH2 = HPC * DH    # 128
NKT = L // 128    # 16 k tiles
QH = 1024         # q chunk for attention phase
NQH = L // QH     # 2
C_LN = -math.log(L)
NEG = -10000.0

F32 = mybir.dt.float32
F32R = mybir.dt.float32r
BF16 = mybir.dt.bfloat16

ATT_DT = BF16     # exp output (PT), V'
MM_DT = F32R      # PE-streamed f32 operands: full rate, ~tf32 rounding

TRACE = False
LAST_RESULT = None

AUX_QM, AUX_KBMC, AUX_ONES, AUX_CLN = 0, 1, 2, 3


def build_nc(with_bias: bool):
    nc = bacc.Bacc(None, target_bir_lowering=False)

    xqT_d = nc.declare_dram_parameter("xqT", [DM, L], MM_DT, isOutput=False)
    xkT_d = nc.declare_dram_parameter("xkT", [DM, L], MM_DT, isOutput=False)
    xvT_d = nc.declare_dram_parameter("xvT", [DM, L], MM_DT, isOutput=False)
    wqs_d = nc.declare_dram_parameter("wqs", [DM, DH2], MM_DT, isOutput=False)
    wks_d = nc.declare_dram_parameter("wks", [DM, DH2], MM_DT, isOutput=False)
    wvs_d = nc.declare_dram_parameter("wvs", [DM, DH2], MM_DT, isOutput=False)
    if with_bias:
        wbias_d = nc.declare_dram_parameter(
            "wbias", [1, 4 * DH2], MM_DT, isOutput=False
        )
    wos_d = nc.declare_dram_parameter("wos", [DH2, DM], MM_DT, isOutput=False)
    aux_d = nc.declare_dram_parameter("aux", [4, L], MM_DT, isOutput=False)
    coef_d = nc.declare_dram_parameter("coef", [1, L], F32, isOutput=False)
    pout_d = nc.declare_dram_parameter("poutT", [DM, L], F32, isOutput=True)

    with tile.TileContext(nc) as tc:
        with (
            tc.tile_pool(name="const", bufs=1) as const,
            tc.tile_pool(name="qek", bufs=1) as qek,
            tc.tile_pool(name="xin", bufs=4) as xin,
            tc.tile_pool(name="vtmp", bufs=1) as vtmp,
            tc.tile_pool(name="ptp", bufs=2) as ptp,
            tc.tile_pool(name="sbB", bufs=1) as sbB,
            tc.tile_pool(name="ps", bufs=1, space="PSUM") as ps,
        ):
            # ---- constants ----
            ident = const.tile([128, 128], F32)
            make_identity(nc, ident)
            wq_sb = const.tile([128, 4, DH2], MM_DT)
            nc.sync.dma_start(
                out=wq_sb, in_=wqs_d[:, :].rearrange("(t p) m -> p t m", p=128)
            )
            wk_sb = const.tile([128, 4, DH2], MM_DT)
            nc.sync.dma_start(
                out=wk_sb, in_=wks_d[:, :].rearrange("(t p) m -> p t m", p=128)
            )
            wv_sb = const.tile([128, 4, DH2], MM_DT)
            nc.sync.dma_start(
                out=wv_sb, in_=wvs_d[:, :].rearrange("(t p) m -> p t m", p=128)
            )
            wo_sb = [const.tile([DH, DM], MM_DT, name=f"wo{h}") for h in range(HPC)]
            for h in range(HPC):
                nc.sync.dma_start(out=wo_sb[h], in_=wos_d[h * DH : (h + 1) * DH, :])
            coef_sb = const.tile([128, NKT], F32)
            nc.sync.dma_start(
                out=coef_sb, in_=coef_d[0:1, :].rearrange("1 (t p) -> p t", p=128)
            )
            if with_bias:
                wb_sb = const.tile([1, 4 * DH2], MM_DT)
                nc.sync.dma_start(out=wb_sb, in_=wbias_d[:, :])
                qm_sb = const.tile([1, L], MM_DT)
                nc.sync.dma_start(out=qm_sb, in_=aux_d[AUX_QM : AUX_QM + 1, :])
                ones_sb = const.tile([1, L], MM_DT)
                nc.sync.dma_start(out=ones_sb, in_=aux_d[AUX_ONES : AUX_ONES + 1, :])

            # ---- persistent per-head operands ----
            QE = [qek.tile([66, L], MM_DT, name=f"QE{h}") for h in range(HPC)]
            KE = [qek.tile([66, L], MM_DT, name=f"KE{h}") for h in range(HPC)]
            Vp = [
                qek.tile([128, NKT, DH + 1], ATT_DT, name=f"Vp{h}") for h in range(HPC)
            ]
            # mask/bias rows of the extended operands (DMA direct from host aux)
            for h in range(HPC):
                nc.sync.dma_start(
                    out=QE[h][64:65, :], in_=aux_d[AUX_QM : AUX_QM + 1, :]
                )
                nc.sync.dma_start(
                    out=QE[h][65:66, :], in_=aux_d[AUX_ONES : AUX_ONES + 1, :]
                )
                nc.sync.dma_start(
                    out=KE[h][64:65, :], in_=aux_d[AUX_KBMC : AUX_KBMC + 1, :]
                )
                nc.sync.dma_start(
                    out=KE[h][65:66, :], in_=aux_d[AUX_CLN : AUX_CLN + 1, :]
                )
                nc.vector.memset(Vp[h][:, :, DH : DH + 1], 1.0)

            def emit_proj(pname, xdram, w_sb, brow, brhs, evict):
                xts = []
                for t in range(4):
                    xt = xin.tile([128, L], MM_DT, tag="xin", name=f"x{pname}{t}")
                    nc.sync.dma_start(out=xt, in_=xdram[t * 128 : (t + 1) * 128, :])
                    xts.append(xt)
                for ch in range(4):
                    sl = slice(ch * 512, (ch + 1) * 512)
                    psp = ps.tile([128, 512], F32, tag="small", bufs=2, name="psp")
                    for t in range(4):
                        nc.tensor.matmul(
                            psp,
                            lhsT=w_sb[:, t, :],
                            rhs=xts[t][:, sl],
                            start=(t == 0),
                            stop=(t == 3 and not with_bias),
                        )
                    if with_bias:
                        nc.tensor.matmul(
                            psp,
                            lhsT=wb_sb[0:1, brow * DH2 : (brow + 1) * DH2],
                            rhs=brhs[0:1, sl],
                            start=False,
                            stop=True,
                        )
                    evict(psp, sl)

            def evict_qk(dst):
                def _e(psp, sl):
                    for h in range(HPC):
                        nc.vector.tensor_copy(
                            out=dst[h][0:DH, sl], in_=psp[h * DH : (h + 1) * DH, :]
                        )

                return _e

            def emit_b1(qh, h):
                """scores + exp for one (q-half, head) -> PT tile"""
                pt = ptp.tile([128, NKT, QH], ATT_DT, tag="pt", name="pt")
                for kt in range(NKT):
                    st = ps.tile([128, QH], F32, tag="st", bufs=2, name="st")
                    for c2 in range(QH // 512):
                        nc.tensor.matmul(
                            st[:, c2 * 512 : (c2 + 1) * 512],
                            lhsT=KE[h][0:66, kt * 128 : (kt + 1) * 128],
                            rhs=QE[h][
                                0:66, qh * QH + c2 * 512 : qh * QH + (c2 + 1) * 512
                            ],
                            start=True,
                            stop=True,
                        )
                    nc.scalar.activation(
                        out=pt[:, kt, :], in_=st, func=mybir.ActivationFunctionType.Exp
                    )
                return pt

            def emit_b2_norm(qh, h, pt):
                """PV + normalization for one (q-half, head) -> normalized O^T"""
                outp = ps.tile([65, QH], F32, tag="outp", bufs=1, name="outp")
                for c2 in range(QH // 512):
                    for kt in range(NKT):
                        nc.tensor.matmul(
                            outp[:, c2 * 512 : (c2 + 1) * 512],
                            lhsT=Vp[h][:, kt, :],
                            rhs=pt[:, kt, c2 * 512 : (c2 + 1) * 512],
                            start=(kt == 0),
                            stop=(kt == NKT - 1),
                        )
                outsb = sbB.tile([65, QH], F32, tag="outsb", name="outsb")
                nc.vector.tensor_copy(out=outsb, in_=outp)
                rcp = sbB.tile([1, QH], F32, tag="rcp", name="rcp")
                nc.vector.reciprocal(out=rcp, in_=outsb[64:65, :])
                rbc = sbB.tile([DH, QH], F32, tag="rbc", name="rbc")
                nc.gpsimd.partition_broadcast(rbc, rcp[0:1, :], channels=DH)
                nr = sbB.tile([DH, QH], MM_DT, tag=f"nrm{h}", name=f"nr{h}")
                nc.vector.tensor_mul(out=nr, in0=outsb[0:DH, :], in1=rbc)
                return nr

            def emit_finals(qh, nrm):
                for dt4 in range(4):
                    for c2 in range(QH // 512):
                        fin = ps.tile([128, 512], F32, tag="small", bufs=2, name="fin")
                        for h in range(HPC):
                            nc.tensor.matmul(
                                fin,
                                lhsT=wo_sb[h][:, dt4 * 128 : (dt4 + 1) * 128],
                                rhs=nrm[h][:, c2 * 512 : (c2 + 1) * 512],
                                start=(h == 0),
                                stop=(h == HPC - 1),
                            )
                        fsb = sbB.tile([128, 512], F32, tag="fsb", bufs=2, name="fsb")
                        nc.vector.tensor_copy(out=fsb, in_=fin)
                        nc.sync.dma_start(
                            out=pout_d[
                                dt4 * 128 : (dt4 + 1) * 128,
                                qh * QH + c2 * 512 : qh * QH + (c2 + 1) * 512,
                            ],
                            in_=fsb,
                        )

            def emit_vproj():
                VT_sb = vtmp.tile([128, L], F32)
                emit_proj(
                    "v",
                    xvT_d,
                    wv_sb,
                    2,
                    ones_sb if with_bias else None,
                    lambda psp, sl: nc.vector.tensor_copy(out=VT_sb[:, sl], in_=psp),
                )
                for kt in range(NKT):
                    tp = ps.tile([128, 128], F32, tag="small", bufs=2, name="tp")
                    nc.tensor.transpose(tp, VT_sb[:, kt * 128 : (kt + 1) * 128], ident)
                    for h in range(HPC):
                        nc.vector.tensor_scalar_mul(
                            out=Vp[h][:, kt, 0:DH],
                            in0=tp[:, h * DH : (h + 1) * DH],
                            scalar1=coef_sb[:, kt : kt + 1],
                        )

            # ---- emission ----
            emit_proj("q", xqT_d, wq_sb, 0, qm_sb if with_bias else None, evict_qk(QE))
            emit_proj(
                "k", xkT_d, wk_sb, 1, ones_sb if with_bias else None, evict_qk(KE)
            )

            pt00 = emit_b1(0, 0)
            emit_vproj()
            nrm0 = [emit_b2_norm(0, 0, pt00)]
            pt01 = emit_b1(0, 1)
            nrm0.append(emit_b2_norm(0, 1, pt01))
            emit_finals(0, nrm0)
            nrm1 = []
            for h in range(HPC):
                pt = emit_b1(1, h)
                nrm1.append(emit_b2_norm(1, h, pt))
            emit_finals(1, nrm1)

    nc.compile()
    return nc


_CACHE = {}


def _get_nc(with_bias: bool):
    key = ("nc", with_bias)
    if key not in _CACHE:
        _CACHE[key] = build_nc(with_bias)
    return _CACHE[key]


def kernel(q, k, v, text_mask, audio_mask, n_head, wq, bq, wk, bk, wv, bv, wo, bo):
    global LAST_RESULT
    q = np.asarray(q, np.float32)
    k = np.asarray(k, np.float32)
    v = np.asarray(v, np.float32)
    text_mask = np.asarray(text_mask, np.float32)
    audio_mask = np.asarray(audio_mask, np.float32)
    wq = np.asarray(wq, np.float32)
    wk = np.asarray(wk, np.float32)
    wv = np.asarray(wv, np.float32)
    wo = np.asarray(wo, np.float32)
    bq = np.asarray(bq, np.float32)
    bk = np.asarray(bk, np.float32)
    bv = np.asarray(bv, np.float32)
    bo = np.asarray(bo, np.float32)
    assert int(n_head) == H

    with_bias = bool(np.any(bq) or np.any(bk) or np.any(bv))

    pad = np.concatenate([text_mask, audio_mask], axis=1)  # [B, L]
    qm = (pad != 0).astype(np.float32)
    tl = text_mask.sum(1)
    al = audio_mask.sum(1)
    tot = tl + al
    coef = np.concatenate(
        [
            text_mask * (tot / (2.0 * tl))[:, None],
            audio_mask * (tot / (2.0 * al))[:, None],
        ],
        axis=1,
    ).astype(np.float32)
    kbmc = (NEG * (1.0 - qm) - C_LN).astype(np.float32)
    ones_row = np.ones((L,), np.float32)
    cln_row = np.full((L,), C_LN, np.float32)

    def cc(a):
        return np.ascontiguousarray(a, dtype=np.float32)

    in_maps = []
    for core in range(NCORES):
        b, hp = divmod(core, NCORES // B)
        cols = slice(hp * DH2, (hp + 1) * DH2)
        m = {
            "xqT": cc((q[b] * qm[b][:, None]).T),
            "xkT": cc(k[b].T),
            "xvT": cc(v[b].T),
            "wqs": cc(wq.T[:, cols]),
            "wks": cc(wk.T[:, cols] / 8.0),
            "wvs": cc(wv.T[:, cols]),
            "wos": cc(wo.T[cols, :]),
            "aux": cc(np.stack([qm[b], kbmc[b], ones_row, cln_row])),
            "coef": cc(coef[b]).reshape(1, L),
        }
        if with_bias:
            m["wbias"] = cc(
                np.concatenate(
                    [bq[cols], bk[cols] / 8.0, bv[cols], np.zeros(DH2, np.float32)]
                )
            ).reshape(1, 4 * DH2)
        in_maps.append(m)

    res = run_bass_kernel_spmd(
        _get_nc(with_bias), in_maps, core_ids=list(range(NCORES)), trace=TRACE
    )
    LAST_RESULT = res

    out = np.zeros((B, L, DM), np.float32)
    npc = NCORES // B
    for b in range(B):
        acc = res.results[b * npc]["poutT"].astype(np.float32).copy()
        for hp in range(1, npc):
            acc += res.results[b * npc + hp]["poutT"]
        out[b] = acc.T + bo[None, :]
    return out


# revision 14
# speedup vs baseline: 1.3104x; 1.1287x over previous
"""Trainium2 Bass kernel for CustomMultiHeadAttention (B=2, L=2048, D=512, H=8).

Sharding: 8 cores = 2 batches x 4 head-pairs. Each core computes, for its
batch b and its 2 heads, the partial output (O_h @ Wo_h summed over its
heads), transposed: poutT [512, 2048]. Host sums the 4 partials per batch,
transposes, and adds bo.

Device-side math per core (all masking folded into matmul contractions):
  Qh = (q[b]*qm) @ WqT_cols + qm*bq_cols          (masked q rows -> exactly 0)
  Kh = k[b] @ WkT_cols/8 + bk_cols/8
  E[k,q] = Kh.Qh + (kb[k]-c)*qm[q] + c            via 2 extra contraction rows
           (kb = -1e4 for masked keys, c = ln(1/2048))
    -> unmasked q: E = s + kb  (masked keys underflow to 0 in exp)
    -> masked q:   E = c       (exp = 1/2048 uniform; denom = 1)
  PT = exp(E)   [k, q] layout
  outT = [Vp | 1]^T @ PT   (Vp = coef * Vh; ones column yields denom row)
  O = outT[0:64] / outT[64]
  poutT[d, q] += Wo_h[:, d] . O[:, q]

Emission order keeps the PE dense (HAM warm): q-proj, k-proj, first
scores+exp unit, then v-proj/transposes, then the remaining attention units.
"""

import math
import os

os.environ.setdefault("MYCRO_LOCAL_CACHE", "1")

import numpy as np

import concourse.bass as bass
import concourse.tile as tile
from concourse import bacc
from concourse import mybir
from concourse.bass_utils import run_bass_kernel_spmd
from concourse.masks import make_identity

B = 2
L = 2048
DM = 512
H = 8
DH = 64
NCORES = 8
HPC = 2           # heads per core
DH2 = HPC * DH    # 128
NKT = L // 128    # 16 k tiles
QH = 1024         # q chunk for attention phase
NQH = L // QH     # 2
C_LN = -math.log(L)
NEG = -10000.0

F32 = mybir.dt.float32
F32R = mybir.dt.float32r
BF16 = mybir.dt.bfloat16

ATT_DT = BF16     # exp output (PT), V'
MM_DT = F32R      # PE-streamed f32 operands: full rate, ~tf32 rounding

TRACE = False
LAST_RESULT = None

AUX_QM, AUX_KBMC, AUX_ONES, AUX_CLN = 0, 1, 2, 3


def build_nc(with_bias: bool):
    nc = bacc.Bacc(None, target_bir_lowering=False)

    xqT_d = nc.declare_dram_parameter("xqT", [DM, L], MM_DT, isOutput=False)
    xkT_d = nc.declare_dram_parameter("xkT", [DM, L], MM_DT, isOutput=False)
    xvT_d = nc.declare_dram_parameter("xvT", [DM, L], MM_DT, isOutput=False)
    wqs_d = nc.declare_dram_parameter("wqs", [DM, DH2], MM_DT, isOutput=False)
    wks_d = nc.declare_dram_parameter("wks", [DM, DH2], MM_DT, isOutput=False)
    wvs_d = nc.declare_dram_parameter("wvs", [DM, DH2], MM_DT, isOutput=False)
    if with_bias:
        wbias_d = nc.declare_dram_parameter(
            "wbias", [1, 4 * DH2], MM_DT, isOutput=False
        )
    wos_d = nc.declare_dram_parameter("wos", [DH2, DM], MM_DT, isOutput=False)
    aux_d = nc.declare_dram_parameter("aux", [4, L], MM_DT, isOutput=False)
    coef_d = nc.declare_dram_parameter("coef", [1, L], F32, isOutput=False)
    pout_d = nc.declare_dram_parameter("poutT", [DM, L], F32, isOutput=True)

    with tile.TileContext(nc) as tc:
        with (
            tc.tile_pool(name="const", bufs=1) as const,
            tc.tile_pool(name="qek", bufs=1) as qek,
            tc.tile_pool(name="xin", bufs=4) as xin,
            tc.tile_pool(name="vtmp", bufs=1) as vtmp,
            tc.tile_pool(name="ptp", bufs=2) as ptp,
            tc.tile_pool(name="sbB", bufs=1) as sbB,
            tc.tile_pool(name="ps", bufs=1, space="PSUM") as ps,
        ):
            # ---- constants ----
            ident = const.tile([128, 128], F32)
            make_identity(nc, ident)
            wq_sb = const.tile([128, 4, DH2], MM_DT)
            nc.sync.dma_start(
                out=wq_sb, in_=wqs_d[:, :].rearrange("(t p) m -> p t m", p=128)
            )
            wk_sb = const.tile([128, 4, DH2], MM_DT)
            nc.sync.dma_start(
                out=wk_sb, in_=wks_d[:, :].rearrange("(t p) m -> p t m", p=128)
            )
            wv_sb = const.tile([128, 4, DH2], MM_DT)
            nc.sync.dma_start(
                out=wv_sb, in_=wvs_d[:, :].rearrange("(t p) m -> p t m", p=128)
            )
            wo_sb = [const.tile([DH, DM], MM_DT, name=f"wo{h}") for h in range(HPC)]
            for h in range(HPC):
                nc.sync.dma_start(out=wo_sb[h], in_=wos_d[h * DH : (h + 1) * DH, :])
            coef_sb = const.tile([128, NKT], F32)
            nc.sync.dma_start(
                out=coef_sb, in_=coef_d[0:1, :].rearrange("1 (t p) -> p t", p=128)
            )
            if with_bias:
                wb_sb = const.tile([1, 4 * DH2], MM_DT)
                nc.sync.dma_start(out=wb_sb, in_=wbias_d[:, :])
                qm_sb = const.tile([1, L], MM_DT)
                nc.sync.dma_start(out=qm_sb, in_=aux_d[AUX_QM : AUX_QM + 1, :])
                ones_sb = const.tile([1, L], MM_DT)
                nc.sync.dma_start(out=ones_sb, in_=aux_d[AUX_ONES : AUX_ONES + 1, :])

            # ---- persistent per-head operands ----
            QE = [qek.tile([66, L], MM_DT, name=f"QE{h}") for h in range(HPC)]
            KE = [qek.tile([66, L], MM_DT, name=f"KE{h}") for h in range(HPC)]
            Vp = [
                qek.tile([128, NKT, DH + 1], ATT_DT, name=f"Vp{h}") for h in range(HPC)
            ]
            # mask/bias rows of the extended operands (DMA direct from host aux)
            for h in range(HPC):
                nc.sync.dma_start(
                    out=QE[h][64:65, :], in_=aux_d[AUX_QM : AUX_QM + 1, :]
                )
                nc.sync.dma_start(
                    out=QE[h][65:66, :], in_=aux_d[AUX_ONES : AUX_ONES + 1, :]
                )
                nc.sync.dma_start(
                    out=KE[h][64:65, :], in_=aux_d[AUX_KBMC : AUX_KBMC + 1, :]
                )
                nc.sync.dma_start(
                    out=KE[h][65:66, :], in_=aux_d[AUX_CLN : AUX_CLN + 1, :]
                )
                nc.vector.memset(Vp[h][:, :, DH : DH + 1], 1.0)

            def emit_proj(pname, xdram, w_sb, brow, brhs, evict):
                xts = []
                for t in range(4):
                    xt = xin.tile([128, L], MM_DT, tag="xin", name=f"x{pname}{t}")
                    nc.sync.dma_start(out=xt, in_=xdram[t * 128 : (t + 1) * 128, :])
                    xts.append(xt)
                for ch in range(4):
                    sl = slice(ch * 512, (ch + 1) * 512)
                    psp = ps.tile([128, 512], F32, tag="small", bufs=2, name="psp")
                    for t in range(4):
                        nc.tensor.matmul(
                            psp,
                            lhsT=w_sb[:, t, :],
                            rhs=xts[t][:, sl],
                            start=(t == 0),
                            stop=(t == 3 and not with_bias),
                        )
                    if with_bias:
                        nc.tensor.matmul(
                            psp,
                            lhsT=wb_sb[0:1, brow * DH2 : (brow + 1) * DH2],
                            rhs=brhs[0:1, sl],
                            start=False,
                            stop=True,
                        )
                    evict(psp, sl)

            def evict_qk(dst):
                def _e(psp, sl):
                    for h in range(HPC):
                        nc.vector.tensor_copy(
                            out=dst[h][0:DH, sl], in_=psp[h * DH : (h + 1) * DH, :]
                        )

                return _e

            def b1_step(qh, h, pt, kt):
                st = ps.tile([128, QH], F32, tag="st", bufs=2, name="st")
                for c2 in range(QH // 512):
                    nc.tensor.matmul(
                        st[:, c2 * 512 : (c2 + 1) * 512],
                        lhsT=KE[h][0:66, kt * 128 : (kt + 1) * 128],
                        rhs=QE[h][
                            0:66, qh * QH + c2 * 512 : qh * QH + (c2 + 1) * 512
                        ],
                        start=True,
                        stop=True,
                    )
                nc.scalar.activation(
                    out=pt[:, kt, :], in_=st, func=mybir.ActivationFunctionType.Exp
                )

            def b1_steps(qh, h, pt):
                for kt in range(NKT):
                    yield lambda kt=kt: b1_step(qh, h, pt, kt)

            def b2_steps(qh, h, pt, outp):
                for kt in range(NKT):
                    def _s(kt=kt):
                        for c2 in range(QH // 512):
                            nc.tensor.matmul(
                                outp[:, c2 * 512 : (c2 + 1) * 512],
                                lhsT=Vp[h][:, kt, :],
                                rhs=pt[:, kt, c2 * 512 : (c2 + 1) * 512],
                                start=(kt == 0),
                                stop=(kt == NKT - 1),
                            )
                    yield _s

            def interleave(*gens):
                gens = [iter(g) for g in gens if g is not None]
                while gens:
                    nxt = []
                    for g in gens:
                        try:
                            next(g)()
                        except StopIteration:
                            continue
                        nxt.append(g)
                    gens = nxt

            def emit_norm(qh, h, outp):
                outsb = sbB.tile([65, QH], F32, tag="outsb", name="outsb")
                nc.vector.tensor_copy(out=outsb, in_=outp)
                rcp = sbB.tile([1, QH], F32, tag="rcp", name="rcp")
                nc.vector.reciprocal(out=rcp, in_=outsb[64:65, :])
                rbc = sbB.tile([DH, QH], F32, tag="rbc", name="rbc")
                nc.gpsimd.partition_broadcast(rbc, rcp[0:1, :], channels=DH)
                nr = sbB.tile([DH, QH], MM_DT, tag=f"nrm{h}", name=f"nr{h}")
                nc.vector.tensor_mul(out=nr, in0=outsb[0:DH, :], in1=rbc)
                return nr

            def emit_finals(qh, nrm):
                for dt4 in range(4):
                    for c2 in range(QH // 512):
                        fin = ps.tile([128, 512], F32, tag="small", bufs=2, name="fin")
                        for h in range(HPC):
                            nc.tensor.matmul(
                                fin,
                                lhsT=wo_sb[h][:, dt4 * 128 : (dt4 + 1) * 128],
                                rhs=nrm[h][:, c2 * 512 : (c2 + 1) * 512],
                                start=(h == 0),
                                stop=(h == HPC - 1),
                            )
                        fsb = sbB.tile([128, 512], F32, tag="fsb", bufs=2, name="fsb")
                        nc.vector.tensor_copy(out=fsb, in_=fin)
                        nc.sync.dma_start(
                            out=pout_d[
                                dt4 * 128 : (dt4 + 1) * 128,
                                qh * QH + c2 * 512 : qh * QH + (c2 + 1) * 512,
                            ],
                            in_=fsb,
                        )

            def vproj_steps():
                VT_sb = vtmp.tile([128, L], F32)
                xts = []
                for t in range(4):
                    xt = xin.tile([128, L], MM_DT, tag="xin", name=f"xv{t}")
                    nc.sync.dma_start(out=xt, in_=xvT_d[t * 128 : (t + 1) * 128, :])
                    xts.append(xt)

                def _chunk(ch):
                    sl = slice(ch * 512, (ch + 1) * 512)
                    psp = ps.tile([128, 512], F32, tag="small", bufs=2, name="psp")
                    for t in range(4):
                        nc.tensor.matmul(
                            psp,
                            lhsT=wv_sb[:, t, :],
                            rhs=xts[t][:, sl],
                            start=(t == 0),
                            stop=(t == 3 and not with_bias),
                        )
                    if with_bias:
                        nc.tensor.matmul(
                            psp,
                            lhsT=wb_sb[0:1, 2 * DH2 : 3 * DH2],
                            rhs=ones_sb[0:1, sl],
                            start=False,
                            stop=True,
                        )
                    nc.vector.tensor_copy(out=VT_sb[:, sl], in_=psp)

                def _tp(kt):
                    tp = ps.tile([128, 128], F32, tag="small", bufs=2, name="tp")
                    nc.tensor.transpose(tp, VT_sb[:, kt * 128 : (kt + 1) * 128], ident)
                    for h in range(HPC):
                        nc.vector.tensor_scalar_mul(
                            out=Vp[h][:, kt, 0:DH],
                            in0=tp[:, h * DH : (h + 1) * DH],
                            scalar1=coef_sb[:, kt : kt + 1],
                        )

                for ch in range(4):
                    yield lambda ch=ch: _chunk(ch)
                for kt in range(NKT):
                    yield lambda kt=kt: _tp(kt)

            # ---- emission: software-pipelined over 4 attention units ----
            emit_proj("q", xqT_d, wq_sb, 0, qm_sb if with_bias else None, evict_qk(QE))
            emit_proj(
                "k", xkT_d, wk_sb, 1, ones_sb if with_bias else None, evict_qk(KE)
            )

            units = [(0, 0), (0, 1), (1, 0), (1, 1)]
            pts = {}
            outps = {}
            nrms = {}
            # unit 0 scores interleaved with the v projection/transpose
            pts[0] = ptp.tile([128, NKT, QH], ATT_DT, tag="pt", name="pt0")
            interleave(b1_steps(0, 0, pts[0]), vproj_steps())
            for i in range(1, 4):
                qh, h = units[i]
                pqh, ph = units[i - 1]
                pts[i] = ptp.tile([128, NKT, QH], ATT_DT, tag="pt", name=f"pt{i}")
                outps[i - 1] = ps.tile([65, QH], F32, tag="outp", bufs=1, name="outp")
                interleave(
                    b1_steps(qh, h, pts[i]),
                    b2_steps(pqh, ph, pts[i - 1], outps[i - 1]),
                )
                nrms[i - 1] = emit_norm(pqh, ph, outps[i - 1])
                if i == 2:
                    emit_finals(0, [nrms[0], nrms[1]])
            outps[3] = ps.tile([65, QH], F32, tag="outp", bufs=1, name="outp")
            for s in b2_steps(1, 1, pts[3], outps[3]):
                s()
            nrms[3] = emit_norm(1, 1, outps[3])
            emit_finals(1, [nrms[2], nrms[3]])

    nc.compile()
    return nc


_CACHE = {}


def _get_nc(with_bias: bool):
    key = ("nc", with_bias)
    if key not in _CACHE:
        _CACHE[key] = build_nc(with_bias)
    return _CACHE[key]


def kernel(q, k, v, text_mask, audio_mask, n_head, wq, bq, wk, bk, wv, bv, wo, bo):
    global LAST_RESULT
    q = np.asarray(q, np.float32)
    k = np.asarray(k, np.float32)
    v = np.asarray(v, np.float32)
    text_mask = np.asarray(text_mask, np.float32)
    audio_mask = np.asarray(audio_mask, np.float32)
    wq = np.asarray(wq, np.float32)
    wk = np.asarray(wk, np.float32)
    wv = np.asarray(wv, np.float32)
    wo = np.asarray(wo, np.float32)
    bq = np.asarray(bq, np.float32)
    bk = np.asarray(bk, np.float32)
    bv = np.asarray(bv, np.float32)
    bo = np.asarray(bo, np.float32)
    assert int(n_head) == H

    with_bias = bool(np.any(bq) or np.any(bk) or np.any(bv))

    pad = np.concatenate([text_mask, audio_mask], axis=1)  # [B, L]
    qm = (pad != 0).astype(np.float32)
    tl = text_mask.sum(1)
    al = audio_mask.sum(1)
    tot = tl + al
    coef = np.concatenate(
        [
            text_mask * (tot / (2.0 * tl))[:, None],
            audio_mask * (tot / (2.0 * al))[:, None],
        ],
        axis=1,
    ).astype(np.float32)
    kbmc = (NEG * (1.0 - qm) - C_LN).astype(np.float32)
    ones_row = np.ones((L,), np.float32)
    cln_row = np.full((L,), C_LN, np.float32)

    def cc(a):
        return np.ascontiguousarray(a, dtype=np.float32)

    in_maps = []
    for core in range(NCORES):
        b, hp = divmod(core, NCORES // B)
        cols = slice(hp * DH2, (hp + 1) * DH2)
        m = {
            "xqT": cc((q[b] * qm[b][:, None]).T),
            "xkT": cc(k[b].T),
            "xvT": cc(v[b].T),
            "wqs": cc(wq.T[:, cols]),
            "wks": cc(wk.T[:, cols] / 8.0),
            "wvs": cc(wv.T[:, cols]),
            "wos": cc(wo.T[cols, :]),
            "aux": cc(np.stack([qm[b], kbmc[b], ones_row, cln_row])),
            "coef": cc(coef[b]).reshape(1, L),
        }
        if with_bias:
            m["wbias"] = cc(
                np.concatenate(
                    [bq[cols], bk[cols] / 8.0, bv[cols], np.zeros(DH2, np.float32)]
                )
            ).reshape(1, 4 * DH2)
        in_maps.append(m)

    res = run_bass_kernel_spmd(
        _get_nc(with_bias), in_maps, core_ids=list(range(NCORES)), trace=TRACE
    )
    LAST_RESULT = res

    out = np.zeros((B, L, DM), np.float32)
    npc = NCORES // B
    for b in range(B):
        acc = res.results[b * npc]["poutT"].astype(np.float32).copy()
        for hp in range(1, npc):
            acc += res.results[b * npc + hp]["poutT"]
        out[b] = acc.T + bo[None, :]
    return out
